# revision 28
# baseline (speedup 1.0000x reference)
"""BoundaryAwareBCELoss Trainium2 kernel (bf16 end-to-end, DMA-cast, lean engines).

loss = mean(w * bce) over (32,1,1024,1024) tensors, where
  bce = -(t*log(p) + (1-t)*log1p(-p)),  t binary
  w   = 3 on the morphological boundary band of t (3x3 dilate - 3x3 erode > 0),
        1 elsewhere.

Math (t in {0,1}):
  x  = |p + t - 1|            ( = p if t==1 else 1-p )   => bce = -ln(x)
  S  = sum over 3x3 window of t (in-image cells only).
  Window uniform (non-boundary) <=> S in {0, 9} for interior pixels,
  tested as q = [|S - 4.5| >= 4].
  w = 3 - 2q   =>   sum(w*bce) = -3*sum(ln x) + 2*sum(q*ln x)

Approximations (all far below the 2e-2 tolerance; ~1e-3 combined):
  * p is cast f32->bf16 during DMA; the Ln input bias (ln(x + 2^-14))
    keeps a p that rounded to exactly 1.0 (x = 0) finite.
  * truncated windows at image borders / 128-row block boundaries keep the
    interior uniformity test, so a few border pixels get the wrong weight.

Sharding: pure data parallel, batch 32 -> 8 cores x 4 images.

Per-core pipeline over groups of 128-row blocks (prologue groups are small
so compute starts early; steady-state groups are 4 blocks = [512,1024]):
  DMA   : t and p cast f32->bf16 in-flight (SWDGE) into zero-padded
          [128, ng, 1028] layouts
  PE    : S = per-block 3x3 window sum: 3 column-shifted matmuls per
          512-col half, banded [128,128] bf16 stationary (vertical window
          via the band, horizontal via rhs shifts)
  ACT   : u = |S - 4.5| (Abs + bias, PSUM->SBUF, 2 blocks per op);
          nl = Ln(x) per group with per-partition accumulate
  DVE   : tm1 = t-1 (4x, exact in bf16) then z = tm1+p (2x, rounds once);
          x = |z| via sign-bit AND (4x mode);
          scr = [u >= 4]*nl per group with accumulate (u pads = 0;
          scalar_tensor_tensor is 1x-only, but splitting q out doesn't
          pay and the fused ISA tensor_tensor_reduce hangs the device)

Host combines the tiny per-partition accumulators:
  loss = (-3*sum(acc_ln) + 2*sum(acc_q)) / N.

Built on Bacc (not plain Bass): its compile pass splits multi-wait
instructions into EventSemaphores to satisfy the 1-wait-per-instruction
hardware limit.
"""

import sys

for _p in ("/opt/trn_rl_repo",):
    if _p not in sys.path:
        sys.path.insert(0, _p)

import numpy as np

import concourse.mybir as mybir
from concourse.bacc import Bacc
from concourse.tile import TileContext
from concourse.bass_utils import run_bass_kernel_spmd

F32 = mybir.dt.float32
BF16 = mybir.dt.bfloat16
U16 = mybir.dt.uint16
ALU = mybir.AluOpType
ACTF = mybir.ActivationFunctionType

B, H, W = 32, 1024, 1024
NCORES = 8
BL = B // NCORES          # images per core
NBLOCKS = BL * H // 128   # 128-row blocks per core = 32
GROUPS = [1, 1, 2] + [4] * 6 + [2, 1, 1]   # prologue/epilogue taper
assert sum(GROUPS) == NBLOCKS
NG = len(GROUPS)
N_TOTAL = B * H * W
PW = W + 4                # padded width; data at cols [2, W+2)
NSUB = NG                 # one accumulator column per group
OUT_COLS = 2 * NSUB       # acc_ln + acc_q


def _consts_np():
    import ml_dtypes

    k = np.arange(128)
    amat = (np.abs(k[:, None] - k[None, :]) <= 1).astype(np.float32)
    return amat.astype(ml_dtypes.bfloat16)


def build_nc():
    nc = Bacc()
    pred_d = nc.dram_tensor("pred", [BL * H, W], F32, kind="ExternalInput")
    targ_d = nc.dram_tensor("target", [BL * H, W], F32, kind="ExternalInput")
    amat_d = nc.dram_tensor("amat", [128, 128], BF16, kind="ExternalInput")
    out_d = nc.dram_tensor("acc_out", [128, OUT_COLS], F32, kind="ExternalOutput")

    with TileContext(nc) as tc:
        with (
            tc.tile_pool(name="const", bufs=1) as const_pool,
            tc.tile_pool(name="zz", bufs=2) as z_pool,
            tc.tile_pool(name="tm1", bufs=2) as tm1_pool,
            tc.tile_pool(name="xx", bufs=2) as x_pool,
            tc.tile_pool(name="nl", bufs=2) as nl_pool,
            tc.tile_pool(name="scr", bufs=2) as scr_pool,
            tc.tile_pool(name="psum", bufs=2, space="PSUM") as psum_pool,
        ):
            a_tile = const_pool.tile([128, 128], BF16)
            nc.sync.dma_start(a_tile[:, :], amat_d[:, :])

            bias_tile = const_pool.tile([128, 1], F32)
            nc.vector.memset(bias_tile[:, :], -4.5)
            # Ln input bias: ln(x + 2^-14) keeps a p that rounded to exactly
            # 1.0 (x = 0) finite; ~1e-3 relative effect on the loss.
            eps_tile = const_pool.tile([128, 1], F32)
            nc.vector.memset(eps_tile[:, :], 2.0**-14)

            acc = const_pool.tile([128, OUT_COLS], F32)
            acc_ln = acc[:, 0:NSUB]
            acc_q = acc[:, NSUB:OUT_COLS]

            # explicit rotating input/u buffers: pad columns are zeroed
            # ONCE here, so no group DMA ever waits on a memset and the
            # per-group Vector memsets disappear.
            tb_bufs = [const_pool.tile([128, 4 * PW], BF16, name=f"tbb{i}") for i in range(5)]
            pb_bufs = [const_pool.tile([128, 4 * PW], BF16, name=f"pbb{i}") for i in range(5)]
            u_bufs = [const_pool.tile([128, 4 * PW], BF16, name=f"ubb{i}") for i in range(3)]
            for buf in tb_bufs + pb_bufs + u_bufs:
                b3 = buf.rearrange("p (n w) -> p n w", n=4)
                nc.vector.memset(b3[:, :, 0:2], 0.0)
                nc.vector.memset(b3[:, :, W + 2 : PW], 0.0)

            r0 = 0
            sub = 0
            for g, ng in enumerate(GROUPS):
                gw = ng * PW

                # t and p: f32 -> bf16 cast during DMA (SWDGE), padded layout
                tb = tb_bufs[g % 5]
                tb3 = tb.rearrange("p (n w) -> p n w", n=4)[:, 0:ng]
                nc.gpsimd.dma_start(
                    tb3[:, :, 2 : W + 2],
                    targ_d[r0 : r0 + ng * 128, :].rearrange(
                        "(n r) w -> r n w", r=128
                    ),
                )

                pb = pb_bufs[g % 5]
                nc.gpsimd.dma_start(
                    pb.rearrange("p (n w) -> p n w", n=4)[:, 0:ng, 2 : W + 2],
                    pred_d[r0 : r0 + ng * 128, :].rearrange(
                        "(n r) w -> r n w", r=128
                    ),
                )

                # group tiles (matmul/u run per 2-block subgroup; DVE and
                # Ln ops are group-wide — finer splits measured slower)
                z = z_pool.tile([128, 4 * PW], BF16, tag="zz")
                tm1 = tm1_pool.tile([128, 4 * PW], BF16, tag="tm1")
                x = x_pool.tile([128, 4 * PW], BF16, tag="xx")
                nl = nl_pool.tile([128, 4 * PW], BF16, tag="nl")
                u = u_bufs[g % 3]
                u3 = u.rearrange("p (n w) -> p n w", n=4)[:, 0:ng]
                scr = scr_pool.tile([128, 4 * PW], BF16, tag="scr")

                # z = (t-1) + p: t-1 is exact in bf16 (tensor_scalar, 4x
                # mode); the tensor_tensor (2x) then rounds the sum once.
                # scalar_tensor_tensor is avoided — it only has a 1x uop.
                nc.vector.tensor_scalar(
                    tm1[:, 0:gw], tb[:, 0:gw], -1.0, None, ALU.add
                )
                nc.vector.tensor_tensor(
                    z[:, 0:gw], tm1[:, 0:gw], pb[:, 0:gw], ALU.add
                )
                # x = |z|: sign-bit clear (4x mode)
                nc.vector.tensor_scalar(
                    x[:, 0:gw].bitcast(U16),
                    z[:, 0:gw].bitcast(U16),
                    0x7FFF,
                    None,
                    ALU.bitwise_and,
                )
                # nl = ln(x + eps); accumulate (pads ~ ln(1)=0)
                nc.scalar.activation(
                    nl[:, 0:gw], x[:, 0:gw], ACTF.Ln,
                    bias=eps_tile[:, :],
                    accum_out=acc_ln[:, sub : sub + 1],
                )

                for n0 in range(0, ng, 2):
                    nb = min(2, ng - n0)
                    # S = 3x3 window sum of t (banded stationary x 3 shifts)
                    S = psum_pool.tile([128, 2 * W], F32, tag="psum")
                    for n in range(n0, n0 + nb):
                        for c in (0, 512):
                            for dj in range(3):
                                nc.tensor.matmul(
                                    S[:, (n - n0) * W + c : (n - n0) * W + c + 512],
                                    a_tile[:, :],
                                    tb3[:, n, 1 + c + dj : 1 + c + dj + 512],
                                    start=(dj == 0),
                                    stop=(dj == 2),
                                )
                    # u = |S - 4.5| (pads stay 0 -> q=0)
                    nc.scalar.activation(
                        u3[:, n0 : n0 + nb, 2 : W + 2],
                        S[:, 0 : nb * W].rearrange("p (n w) -> p n w", n=nb),
                        ACTF.Abs,
                        bias=bias_tile[:, :],
                    )

                # sum(q * ln x), q = [u >= 4]
                nc.vector.scalar_tensor_tensor(
                    scr[:, 0:gw],
                    u[:, 0:gw],
                    4.0,
                    nl[:, 0:gw],
                    ALU.is_ge,
                    ALU.mult,
                    accum_out=acc_q[:, sub : sub + 1],
                )
                sub += 1

                r0 += ng * 128

            nc.sync.dma_start(out_d[:, :], acc[:, :])

    nc.finalize()
    return nc


_NC_CACHE = None


def _get_nc():
    global _NC_CACHE
    if _NC_CACHE is None:
        _NC_CACHE = build_nc()
    return _NC_CACHE


def run_spmd(pred, target, **kwargs):
    """Shard, run on 8 cores, return BassKernelResults."""
    pred = np.asarray(pred, dtype=np.float32).reshape(B * H, W)
    target = np.asarray(target, dtype=np.float32).reshape(B * H, W)
    amat = _consts_np()
    in_maps = []
    for i in range(NCORES):
        sl = slice(i * BL * H, (i + 1) * BL * H)
        in_maps.append(
            {
                "pred": np.ascontiguousarray(pred[sl]),
                "target": np.ascontiguousarray(target[sl]),
                "amat": amat,
            }
        )
    nc = _get_nc()
    return run_bass_kernel_spmd(nc, in_maps, core_ids=list(range(NCORES)), **kwargs)


def combine(results):
    s_ln = 0.0
    s_q = 0.0
    for r in results:
        acc = np.asarray(r["acc_out"], dtype=np.float64)
        s_ln += acc[:, 0:NSUB].sum()
        s_q += acc[:, NSUB:].sum()
    loss = (-3.0 * s_ln + 2.0 * s_q) / N_TOTAL
    return np.array(loss, dtype=np.float32)


def kernel(pred, target):
    res = run_spmd(pred, target)
    return combine(res.results)


# revision 29
# speedup vs baseline: 1.0096x; 1.0096x over previous
"""BoundaryAwareBCELoss Trainium2 kernel (bf16 end-to-end, DMA-cast, lean engines).

loss = mean(w * bce) over (32,1,1024,1024) tensors, where
  bce = -(t*log(p) + (1-t)*log1p(-p)),  t binary
  w   = 3 on the morphological boundary band of t (3x3 dilate - 3x3 erode > 0),
        1 elsewhere.

Math (t in {0,1}):
  x  = |p + t - 1|            ( = p if t==1 else 1-p )   => bce = -ln(x)
  S  = sum over 3x3 window of t (in-image cells only).
  Window uniform (non-boundary) <=> S in {0, 9} for interior pixels,
  tested as q = [|S - 4.5| >= 4].
  w = 3 - 2q   =>   sum(w*bce) = -3*sum(ln x) + 2*sum(q*ln x)

Approximations (all far below the 2e-2 tolerance; ~1e-3 combined):
  * p is cast f32->bf16 during DMA; the Ln input bias (ln(x + 2^-14))
    keeps a p that rounded to exactly 1.0 (x = 0) finite.
  * truncated windows at image borders / 128-row block boundaries keep the
    interior uniformity test, so a few border pixels get the wrong weight.

Sharding: pure data parallel, batch 32 -> 8 cores x 4 images.

Per-core pipeline over groups of 128-row blocks (prologue groups are small
so compute starts early; steady-state groups are 4 blocks = [512,1024]):
  DMA   : t and p cast f32->bf16 in-flight (SWDGE) into zero-padded
          [128, ng, 1028] layouts
  PE    : S = per-block 3x3 window sum: 3 column-shifted matmuls per
          512-col half, banded [128,128] bf16 stationary (vertical window
          via the band, horizontal via rhs shifts)
  ACT   : u = |S - 4.5| (Abs + bias, PSUM->SBUF, 2 blocks per op);
          nl = Ln(x) per group with per-partition accumulate
  DVE   : tm1 = t-1 (4x, exact in bf16) then z = tm1+p (2x, rounds once);
          x = |z| via sign-bit AND (4x mode);
          scr = [u >= 4]*nl per group with accumulate (u pads = 0;
          scalar_tensor_tensor is 1x-only, but splitting q out doesn't
          pay and the fused ISA tensor_tensor_reduce hangs the device)

Host combines the tiny per-partition accumulators:
  loss = (-3*sum(acc_ln) + 2*sum(acc_q)) / N.

Built on Bacc (not plain Bass): its compile pass splits multi-wait
instructions into EventSemaphores to satisfy the 1-wait-per-instruction
hardware limit.
"""

import sys

for _p in ("/opt/trn_rl_repo",):
    if _p not in sys.path:
        sys.path.insert(0, _p)

import numpy as np

import concourse.mybir as mybir
from concourse.bacc import Bacc
from concourse.tile import TileContext
from concourse.bass_utils import run_bass_kernel_spmd

F32 = mybir.dt.float32
BF16 = mybir.dt.bfloat16
U16 = mybir.dt.uint16
ALU = mybir.AluOpType
ACTF = mybir.ActivationFunctionType

B, H, W = 32, 1024, 1024
NCORES = 8
BL = B // NCORES          # images per core
NBLOCKS = BL * H // 128   # 128-row blocks per core = 32
GROUPS = [1, 1, 2] + [4] * 6 + [2, 1, 1]   # prologue/epilogue taper
assert sum(GROUPS) == NBLOCKS
NG = len(GROUPS)
N_TOTAL = B * H * W
PW = W + 4                # padded width; data at cols [2, W+2)
NSUB = NG                 # one accumulator column per group
OUT_COLS = 2 * NSUB       # acc_ln + acc_q


def _consts_np():
    import ml_dtypes

    k = np.arange(128)
    amat = (np.abs(k[:, None] - k[None, :]) <= 1).astype(np.float32)
    return amat.astype(ml_dtypes.bfloat16)


def build_nc():
    nc = Bacc()
    pred_d = nc.dram_tensor("pred", [BL * H, W], F32, kind="ExternalInput")
    targ_d = nc.dram_tensor("target", [BL * H, W], F32, kind="ExternalInput")
    amat_d = nc.dram_tensor("amat", [128, 128], BF16, kind="ExternalInput")
    out_d = nc.dram_tensor("acc_out", [128, OUT_COLS], F32, kind="ExternalOutput")

    with TileContext(nc) as tc:
        with (
            tc.tile_pool(name="const", bufs=1) as const_pool,
            tc.tile_pool(name="zz", bufs=2) as z_pool,
            tc.tile_pool(name="tm1", bufs=2) as tm1_pool,
            tc.tile_pool(name="xx", bufs=2) as x_pool,
            tc.tile_pool(name="nl", bufs=2) as nl_pool,
            tc.tile_pool(name="scr", bufs=2) as scr_pool,
            tc.tile_pool(name="psum", bufs=4, space="PSUM") as psum_pool,
        ):
            a_tile = const_pool.tile([128, 128], BF16)
            nc.sync.dma_start(a_tile[:, :], amat_d[:, :])

            bias_tile = const_pool.tile([128, 1], F32)
            nc.vector.memset(bias_tile[:, :], -4.5)
            # Ln input bias: ln(x + 2^-14) keeps a p that rounded to exactly
            # 1.0 (x = 0) finite; ~1e-3 relative effect on the loss.
            eps_tile = const_pool.tile([128, 1], F32)
            nc.vector.memset(eps_tile[:, :], 2.0**-14)

            acc = const_pool.tile([128, OUT_COLS], F32)
            acc_ln = acc[:, 0:NSUB]
            acc_q = acc[:, NSUB:OUT_COLS]

            # explicit rotating input/u buffers: pad columns are zeroed
            # ONCE here, so no group DMA ever waits on a memset and the
            # per-group Vector memsets disappear.
            tb_bufs = [const_pool.tile([128, 4 * PW], BF16, name=f"tbb{i}") for i in range(4)]
            pb_bufs = [const_pool.tile([128, 4 * PW], BF16, name=f"pbb{i}") for i in range(4)]
            u_bufs = [const_pool.tile([128, 4 * PW], BF16, name=f"ubb{i}") for i in range(3)]
            for buf in tb_bufs + pb_bufs + u_bufs:
                b3 = buf.rearrange("p (n w) -> p n w", n=4)
                nc.vector.memset(b3[:, :, 0:2], 0.0)
                nc.vector.memset(b3[:, :, W + 2 : PW], 0.0)

            r0 = 0
            sub = 0
            for g, ng in enumerate(GROUPS):
                gw = ng * PW

                # t and p: f32 -> bf16 cast during DMA (SWDGE), padded layout
                tb = tb_bufs[g % 4]
                tb3 = tb.rearrange("p (n w) -> p n w", n=4)[:, 0:ng]
                nc.gpsimd.dma_start(
                    tb3[:, :, 2 : W + 2],
                    targ_d[r0 : r0 + ng * 128, :].rearrange(
                        "(n r) w -> r n w", r=128
                    ),
                )

                pb = pb_bufs[g % 4]
                nc.gpsimd.dma_start(
                    pb.rearrange("p (n w) -> p n w", n=4)[:, 0:ng, 2 : W + 2],
                    pred_d[r0 : r0 + ng * 128, :].rearrange(
                        "(n r) w -> r n w", r=128
                    ),
                )

                # group tiles (matmul/u run per 2-block subgroup; DVE and
                # Ln ops are group-wide — finer splits measured slower)
                z = z_pool.tile([128, 4 * PW], BF16, tag="zz")
                tm1 = tm1_pool.tile([128, 4 * PW], BF16, tag="tm1")
                x = x_pool.tile([128, 4 * PW], BF16, tag="xx")
                nl = nl_pool.tile([128, 4 * PW], BF16, tag="nl")
                u = u_bufs[g % 3]
                u3 = u.rearrange("p (n w) -> p n w", n=4)[:, 0:ng]
                scr = scr_pool.tile([128, 4 * PW], BF16, tag="scr")

                # z = (t-1) + p: t-1 is exact in bf16 (tensor_scalar, 4x
                # mode); the tensor_tensor (2x) then rounds the sum once.
                # scalar_tensor_tensor is avoided — it only has a 1x uop.
                nc.vector.tensor_scalar(
                    tm1[:, 0:gw], tb[:, 0:gw], -1.0, None, ALU.add
                )
                nc.vector.tensor_tensor(
                    z[:, 0:gw], tm1[:, 0:gw], pb[:, 0:gw], ALU.add
                )
                # x = |z|: sign-bit clear (4x mode)
                nc.vector.tensor_scalar(
                    x[:, 0:gw].bitcast(U16),
                    z[:, 0:gw].bitcast(U16),
                    0x7FFF,
                    None,
                    ALU.bitwise_and,
                )
                # nl = ln(x + eps); accumulate (pads ~ ln(1)=0)
                nc.scalar.activation(
                    nl[:, 0:gw], x[:, 0:gw], ACTF.Ln,
                    bias=eps_tile[:, :],
                    accum_out=acc_ln[:, sub : sub + 1],
                )

                for n in range(ng):
                    # S = 3x3 window sum of t (banded stationary x 3 shifts)
                    S = psum_pool.tile([128, W], F32, tag="psum")
                    for c in (0, 512):
                        for dj in range(3):
                            nc.tensor.matmul(
                                S[:, c : c + 512],
                                a_tile[:, :],
                                tb3[:, n, 1 + c + dj : 1 + c + dj + 512],
                                start=(dj == 0),
                                stop=(dj == 2),
                            )
                    # u = |S - 4.5| (pads stay 0 -> q=0)
                    nc.scalar.activation(
                        u3[:, n, 2 : W + 2],
                        S[:, :],
                        ACTF.Abs,
                        bias=bias_tile[:, :],
                    )

                # sum(q * ln x), q = [u >= 4]
                nc.vector.scalar_tensor_tensor(
                    scr[:, 0:gw],
                    u[:, 0:gw],
                    4.0,
                    nl[:, 0:gw],
                    ALU.is_ge,
                    ALU.mult,
                    accum_out=acc_q[:, sub : sub + 1],
                )
                sub += 1

                r0 += ng * 128

            nc.sync.dma_start(out_d[:, :], acc[:, :])

    nc.finalize()
    return nc


_NC_CACHE = None


def _get_nc():
    global _NC_CACHE
    if _NC_CACHE is None:
        _NC_CACHE = build_nc()
    return _NC_CACHE


def run_spmd(pred, target, **kwargs):
    """Shard, run on 8 cores, return BassKernelResults."""
    pred = np.asarray(pred, dtype=np.float32).reshape(B * H, W)
    target = np.asarray(target, dtype=np.float32).reshape(B * H, W)
    amat = _consts_np()
    in_maps = []
    for i in range(NCORES):
        sl = slice(i * BL * H, (i + 1) * BL * H)
        in_maps.append(
            {
                "pred": np.ascontiguousarray(pred[sl]),
                "target": np.ascontiguousarray(target[sl]),
                "amat": amat,
            }
        )
    nc = _get_nc()
    return run_bass_kernel_spmd(nc, in_maps, core_ids=list(range(NCORES)), **kwargs)


def combine(results):
    s_ln = 0.0
    s_q = 0.0
    for r in results:
        acc = np.asarray(r["acc_out"], dtype=np.float64)
        s_ln += acc[:, 0:NSUB].sum()
        s_q += acc[:, NSUB:].sum()
    loss = (-3.0 * s_ln + 2.0 * s_q) / N_TOTAL
    return np.array(loss, dtype=np.float32)


def kernel(pred, target):
    res = run_spmd(pred, target)
    return combine(res.results)
